# revision 1
# baseline (speedup 1.0000x reference)
"""Trainium2 Bass kernel for nn_Coupling: out[e, s*J+j] = sum_a feat[e, a*S+s] * P[a, j].

Sharding: env axis data-parallel across 8 cores (3750 envs/core); P is tiny and
built host-side, replicated to every core as a [108, 30] block-diagonal input.

Per-core device kernel:
  - K-packs 3 envs into one matmul contraction (K = 3*36 = 108 partitions),
    with features as the stationary operand and the 3-env block-diagonal P as
    the 30-column moving operand.
  - For each env-triple, G matmuls: matmul r uses feature columns s = r (mod G),
    so output partition m holds s = G*m + r.  Per partition the (s_lo, j) block
    is G*10 floats contiguous and 64B-aligned in DRAM, keeping the output DMA
    efficient despite the s-major/j-minor interleaved output layout.
  - 128/(S/G) triples share one PSUM batch (col-tiling); one DVE copy per batch
    permutes (r,t,j) -> (t,r,j) into an SBUF staging buffer; one large DMA per
    partition-quarter flushes each stage.
"""

import math

import numpy as np

import concourse.mybir as mybir
from concourse import bacc, tile
from concourse.bass_utils import run_bass_kernel_spmd

ENV = 30000
A = 36          # n_alpha
S = 256         # soap
J = 10          # n_j
N_CORES = 8
E_CORE = ENV // N_CORES  # 3750

T = 3           # envs packed into one matmul contraction (K = T*A = 108)
G = 8           # s values interleaved per output partition (run = G*J*4 bytes)
NT_LD = 4       # triples per feature-load DMA
NB = 8          # PSUM batches per output stage

F32 = mybir.dt.float32

_NC_CACHE = {}


def build_nc(n_env, g=G, fbufs=12, stbufs=2, psbufs=8, dma_only=False):
    assert n_env % T == 0
    n_tri = n_env // T
    part = S // g           # output partitions per triple
    nt_ps = 128 // part     # triples per PSUM batch
    st_tri = NB * nt_ps     # triples per stage

    nc = bacc.Bacc("TRN2", target_bir_lowering=False, debug=False)

    feat = nc.dram_tensor("features", [n_env, A * S], F32, kind="ExternalInput")
    pblk = nc.dram_tensor("pblk", [T * A, T * J], F32, kind="ExternalInput")
    out = nc.dram_tensor("out", [n_env, S * J], F32, kind="ExternalOutput")

    feat3 = feat.rearrange("e (a s) -> e a s", a=A)
    out3 = out.rearrange("e (sh x) -> e sh x", sh=part)  # x = s_lo*J + j

    with tile.TileContext(nc) as tc:
        with (
            tc.tile_pool(name="const", bufs=1) as cpool,
            tc.tile_pool(name="feat", bufs=fbufs) as fpool,
            tc.tile_pool(name="psum", bufs=psbufs, space="PSUM") as pspool,
            tc.tile_pool(name="stage", bufs=stbufs) as stpool,
        ):
            pb = cpool.tile([T * A, T * J], F32)
            nc.sync.dma_start(pb[:], pblk[:])
            dummy = None
            if dma_only:
                dummy = cpool.tile([128, NB, T, g * J], F32)
                nc.gpsimd.memset(dummy[:], 0.0)

            tri0 = 0
            while tri0 < n_tri:
                n_tri_st = min(st_tri, n_tri - tri0)
                n_grp = math.ceil(n_tri_st / NT_LD)
                e0 = tri0 * T

                # load feature groups (NT_LD consecutive triples each)
                fts = []
                for gi in range(n_grp):
                    nt = min(NT_LD, n_tri_st - gi * NT_LD)
                    eg = e0 + gi * NT_LD * T
                    ft = fpool.tile([T * A, nt, S], F32)
                    nc.sync.dma_start(
                        ft[:],
                        feat3[eg : eg + nt * T].rearrange(
                            "(m t) a s -> t a m s", t=T
                        ),
                    )
                    fts.append(ft.rearrange("p m (sh g) -> p m g sh", g=g))

                if not dma_only:
                    stage = stpool.tile([128, NB, T, g * J], F32)
                    # triple tau -> quarter q = tau//NB, psum batch b = tau%NB
                    for b in range(min(NB, n_tri_st)):
                        nq = sum(1 for q in range(nt_ps) if NB * q + b < n_tri_st)
                        ps = pspool.tile([128, g, T, J], F32)
                        for q in range(nq):
                            tau = NB * q + b
                            gi, mm = divmod(tau, NT_LD)
                            for r in range(g):
                                nc.tensor.matmul(
                                    ps[q * part : (q + 1) * part, r],
                                    fts[gi][:, mm, r],
                                    pb[:],
                                    tile_position=(0, q * part),
                                )
                        nc.vector.tensor_copy(
                            stage[: nq * part, b].rearrange(
                                "p t (r j) -> p t r j", r=g
                            ),
                            ps[: nq * part].rearrange("p r t j -> p t r j"),
                        )
                else:
                    stage = dummy

                # flush stage: quarter q covers triples [NB*q, NB*q+NB)
                for q in range(math.ceil(n_tri_st / NB)):
                    nb_q = min(NB, n_tri_st - NB * q)
                    eq = e0 + NB * q * T
                    nc.scalar.dma_start(
                        out3[eq : eq + nb_q * T].rearrange("e sh x -> sh e x"),
                        stage[q * part : (q + 1) * part, :nb_q],
                    )

                tri0 += n_tri_st

    nc.compile()
    return nc


def _get_nc(n_env, **kw):
    key = (n_env, tuple(sorted(kw.items())))
    if key not in _NC_CACHE:
        _NC_CACHE[key] = build_nc(n_env, **kw)
    return _NC_CACHE[key]


def make_pblk(U, alpha1, alpha2, j1, j2):
    P = (U[alpha1][:, j1] * U[alpha2][:, j2]).astype(np.float32)  # [A, J]
    pblk = np.zeros((T * A, T * J), dtype=np.float32)
    for t in range(T):
        pblk[t * A : (t + 1) * A, t * J : (t + 1) * J] = P
    return pblk


def run_spmd(features, U, alpha1, alpha2, j1, j2, trace=False, **kw):
    features = np.asarray(features, dtype=np.float32)
    pblk = make_pblk(
        np.asarray(U), np.asarray(alpha1), np.asarray(alpha2),
        np.asarray(j1), np.asarray(j2),
    )
    nc = _get_nc(E_CORE, **kw)
    in_maps = [
        {"features": features[c * E_CORE : (c + 1) * E_CORE], "pblk": pblk}
        for c in range(N_CORES)
    ]
    res = run_bass_kernel_spmd(nc, in_maps, list(range(N_CORES)), trace=trace)
    out = np.concatenate([res.results[c]["out"] for c in range(N_CORES)], axis=0)
    return out, res


def kernel(features, U, alpha1, alpha2, j1, j2):
    return run_spmd(features, U, alpha1, alpha2, j1, j2)[0]



# revision 2
# speedup vs baseline: 1.0932x; 1.0932x over previous
"""Trainium2 Bass kernel for nn_Coupling: out[e, s*J+j] = sum_a feat[e, a*S+s] * P[a, j].

Sharding: env axis data-parallel across 8 cores (3750 envs/core); P is tiny and
built host-side, replicated to every core as a [108, 32] block-diagonal input
(cols 30:32 zero-padded).

Per-core device kernel (v2 — P-stationary streaming + PE transpose):
  - K-packs 3 envs (K = 3*36 = 108).  P_blk is the STATIONARY operand (32 cols,
    reloaded per matmul but only 32-col LDW); features stream as the moving
    operand in 512-col fp32r chunks (1 cyc/col at N>=256), so TensorE cost is
    ~N_cols total instead of per-tiny-matmul overhead.
  - 4 matmuls with tile_position=(0,32q) fill all 128 PSUM partitions (4 octets
    of 8 triples each), so the PSUM->SBUF copy runs full-lane on DVE.
  - A second TensorE pass transposes [128,128] blocks (s_lo-sliced) so the
    final layout is [(tr,s_hi) partitions, (q,t,s_lo,j) cols]: output DMA runs
    are (s_lo,j) = 160 f32 = 640B contiguous in DRAM — at/above the 512B
    HBM line-rate threshold.
"""

import numpy as np

import concourse.mybir as mybir
from concourse import bacc, tile
from concourse.bass_utils import run_bass_kernel_spmd

ENV = 30000
A = 36          # n_alpha
S = 256         # soap
J = 10          # n_j
N_CORES = 8
E_CORE = ENV // N_CORES  # 3750

T = 3           # envs packed into one matmul contraction (K = T*A = 108)
K = T * A       # 108
M = 32          # stationary free size: (t,j) = 30 cols + 2 zero pad
TR = 8          # triples per octet
Q = 4           # octets per superblock (fills 128 psum partitions)
SB_TRI = Q * TR  # 32 triples / superblock = 96 envs
SH = 16         # s_hi values (s = sh*16 + sl)
SL = 16         # s_lo values

F32 = mybir.dt.float32
BF16 = mybir.dt.bfloat16

_NC_CACHE = {}


def build_nc(n_env, fbufs=12, x4bufs=3, ps1bufs=3, ps2bufs=2, stbufs=3):
    assert n_env % T == 0
    n_tri = n_env // T
    nsb = n_tri // SB_TRI
    tail_tri = n_tri - nsb * SB_TRI   # leftover triples (handled Q=1 path)
    assert tail_tri * T + nsb * SB_TRI * T == n_env

    nc = bacc.Bacc("TRN2", target_bir_lowering=False, debug=False)

    feat = nc.dram_tensor("features", [n_env, A * S], F32, kind="ExternalInput")
    pblk = nc.dram_tensor("pblk", [K, M], F32, kind="ExternalInput")
    ident = nc.dram_tensor("ident", [128, 128], F32, kind="ExternalInput")
    out = nc.dram_tensor("out", [n_env, S * J], F32, kind="ExternalOutput")

    feat3 = feat.rearrange("e (a s) -> e a s", a=A)

    with tile.TileContext(nc) as tc:
        with (
            tc.tile_pool(name="const", bufs=1) as cpool,
            tc.tile_pool(name="feat", bufs=fbufs) as fpool,
            tc.tile_pool(name="x4", bufs=x4bufs) as x4pool,
            tc.tile_pool(name="ps1", bufs=ps1bufs, space="PSUM") as ps1pool,
            tc.tile_pool(name="ps2", bufs=ps2bufs, space="PSUM") as ps2pool,
            tc.tile_pool(name="stage", bufs=stbufs) as stpool,
        ):
            pb = cpool.tile([K, M], BF16)
            nc.gpsimd.dma_start(pb[:], pblk[:])
            idt = cpool.tile([128, 128], F32)
            nc.sync.dma_start(idt[:], ident[:])

            for sb in range(nsb):
                tri0 = sb * SB_TRI
                e0 = tri0 * T

                # Octets interleave triples: tri_local = tr*Q + q, so for a
                # fixed tr the 12 envs (q,t) are consecutive in DRAM and the
                # output DMA per tr is a 3-dim AP with 640B runs.
                fsrc = feat3[e0 : e0 + SB_TRI * T].rearrange(
                    "(tr q t) a s -> q (t a) tr s", q=Q, t=T
                )
                fts = []
                for q in range(Q):
                    ft = fpool.tile([K, TR, S], BF16)
                    nc.gpsimd.dma_start(ft[:], fsrc[q])
                    fts.append(ft)

                # 4 col-strip matmuls per 512-col chunk -> full 128-partition
                # psum bank; one full-lane DVE copy into x4.
                x4 = x4pool.tile([128, TR * S], F32)
                for p in range(TR * S // 512):
                    ps1 = ps1pool.tile([128, 512], F32)
                    for q in range(Q):
                        nc.tensor.matmul(
                            ps1[32 * q : 32 * q + 32, :],
                            pb[:],
                            fts[q].rearrange("k m s -> k (m s)")[
                                :, p * 512 : (p + 1) * 512
                            ],
                            tile_position=(0, 32 * q),
                        )
                    nc.vector.tensor_copy(
                        x4[:, p * 512 : (p + 1) * 512], ps1[:]
                    )

                # PE transpose: [128 parts=(q,t*10+j), 128 cols=(tr,sh)] ->
                # psum2[(tr,sh), (q, t*10+j)] per s_lo slice.
                xv = x4.rearrange("p (tr sh sl) -> p sl (tr sh)", sh=SH, sl=SL)
                stage = stpool.tile([128, Q, T, SL, J], F32)
                for h in range(2):
                    ps2 = ps2pool.tile([128, 8, 128], F32)
                    for sl in range(8):
                        slo = h * 8 + sl
                        nc.tensor.transpose(
                            ps2[:, sl, :], xv[:, slo, :], idt[:]
                        )
                    pv = ps2.rearrange("p sl (q r) -> p q sl r", q=Q)
                    for t in range(T):
                        nc.vector.tensor_copy(
                            stage[:, :, t, 8 * h : 8 * h + 8, :],
                            pv[:, :, :, t * J : (t + 1) * J],
                        )

                for tr in range(TR):
                    eg = e0 + tr * Q * T
                    (nc.scalar if tr % 2 == 0 else nc.sync).dma_start(
                        out[eg : eg + Q * T].rearrange(
                            "e (sh x) -> sh e x", sh=SH
                        ),
                        stage[SH * tr : SH * (tr + 1)].rearrange(
                            "sh q t sl j -> sh (q t) (sl j)"
                        ),
                    )

            if tail_tri:
                # Small path: Q=1, TR=tail_tri (e.g. 2 triples = 6 envs).
                tr_t = tail_tri
                tri0 = nsb * SB_TRI
                e0 = tri0 * T
                n_e = tr_t * T
                ncol = tr_t * S
                ft = fpool.tile([K, tr_t, S], BF16)
                nc.gpsimd.dma_start(
                    ft[:],
                    feat3[e0 : e0 + n_e].rearrange("(m t) a s -> (t a) m s", t=T),
                )
                x4t = x4pool.tile([32, ncol], F32)
                for p in range(ncol // 512):
                    ps1 = ps1pool.tile([32, 512], F32)
                    nc.tensor.matmul(
                        ps1[:],
                        pb[:],
                        ft.rearrange("k m s -> k (m s)")[
                            :, p * 512 : (p + 1) * 512
                        ],
                    )
                    nc.vector.tensor_copy(x4t[:, p * 512 : (p + 1) * 512], ps1[:])
                npart = tr_t * SH
                xvt = x4t.rearrange("p (tr sh sl) -> p sl (tr sh)", sh=SH, sl=SL)
                ps2 = ps2pool.tile([npart, SL, 32], F32)
                for slo in range(SL):
                    nc.tensor.transpose(
                        ps2[:, slo, :], xvt[:, slo, :], idt[0:32, 0:32]
                    )
                staget = stpool.tile([npart, T, SL, J], F32)
                for t in range(T):
                    nc.vector.tensor_copy(
                        staget[:, t, :, :], ps2[:, :, t * J : (t + 1) * J]
                    )
                for tr in range(tr_t):
                    nc.scalar.dma_start(
                        out[e0 + T * tr : e0 + T * (tr + 1)].rearrange(
                            "e (sh x) -> sh e x", sh=SH
                        ),
                        staget[SH * tr : SH * (tr + 1)].rearrange(
                            "sh t sl j -> sh t (sl j)"
                        ),
                    )

    nc.compile()
    return nc


def _get_nc(n_env, **kw):
    key = (n_env, tuple(sorted(kw.items())))
    if key not in _NC_CACHE:
        _NC_CACHE[key] = build_nc(n_env, **kw)
    return _NC_CACHE[key]


def make_pblk(U, alpha1, alpha2, j1, j2):
    P = (U[alpha1][:, j1] * U[alpha2][:, j2]).astype(np.float32)  # [A, J]
    pblk = np.zeros((K, M), dtype=np.float32)
    for t in range(T):
        pblk[t * A : (t + 1) * A, t * J : (t + 1) * J] = P
    return pblk


def run_spmd(features, U, alpha1, alpha2, j1, j2, trace=False, **kw):
    features = np.asarray(features, dtype=np.float32)
    pblk = make_pblk(
        np.asarray(U), np.asarray(alpha1), np.asarray(alpha2),
        np.asarray(j1), np.asarray(j2),
    )
    ident = np.eye(128, dtype=np.float32)
    nc = _get_nc(E_CORE, **kw)
    in_maps = [
        {
            "features": features[c * E_CORE : (c + 1) * E_CORE],
            "pblk": pblk,
            "ident": ident,
        }
        for c in range(N_CORES)
    ]
    res = run_bass_kernel_spmd(nc, in_maps, list(range(N_CORES)), trace=trace)
    out = np.concatenate([res.results[c]["out"] for c in range(N_CORES)], axis=0)
    return out, res


def kernel(features, U, alpha1, alpha2, j1, j2):
    return run_spmd(features, U, alpha1, alpha2, j1, j2)[0]
